# revision 1
# baseline (speedup 1.0000x reference)
"""Trainium2 Bass kernel: causal MultiHeadAttention with RoPE (head-parallel).

B=1, S=4096, D=768, H=12 heads, dk=64, fp32 I/O. 8 NeuronCores, SPMD.

Sharding: head-parallel with split tail heads. Core c owns whole head
A=c (heads 0..7) plus HALF of head B=8+c//2: the q-tiles of parity c%2
(4 tiles of 256 rows each). Every core projects K/V/Q for its two heads
over the full sequence, runs full-causal attention for head A and its
four B q-tiles (identical instruction stream everywhere; the lone
parity-dependent bit is a [128,1024] mask input), computes bf16 partial
output projections, and combines them with ReduceScatter collectives:
an 8-way RS for the A-partials (q-sharded result) and a 4-way RS over
each parity group for the B-partials, which the host adds on top.
"""

import sys

if "/opt/trn_rl_repo" not in sys.path:
    sys.path.insert(0, "/opt/trn_rl_repo")

import numpy as np
import ml_dtypes

D_MODEL = 768
H = 12
DK = 64
S = 4096
THETA = 10000.0
MAX_SEQ_LEN = 4096
N_CORES = 8
EB = D_MODEL // 128   # 6 contraction blocks
N_CH = S // 512       # 8 sequence chunks
VW = 130              # V_aug row width per s-tile: 2 heads x (64+ones)

BF16 = ml_dtypes.bfloat16

# ReduceScatter chunks: fire after q-tile T_FIRE, covering q rows [lo, hi).
# Fired one q-tile after the covered rows complete so the collective's
# input-DMA waits are already satisfied and never block the Pool queue.
RS_CHUNKS = [
    (10, 0, 2560),
    (13, 2560, 3584),
    (15, 3584, 4096),
]
# out_d row offset of each chunk's shard
RS_OUT_OFF = [0, 320, 448]


def build_program(with_rs=True):
    import concourse.mybir as mybir
    import concourse.tile as tile
    from concourse import bacc, library_config
    from concourse.tile import add_dep_helper

    f32 = mybir.dt.float32
    bf16 = mybir.dt.bfloat16
    Exp = mybir.ActivationFunctionType.Exp
    Copy = mybir.ActivationFunctionType.Copy

    nc = bacc.Bacc(
        "TRN2",
        target_bir_lowering=False,
        debug=False,
        enable_asserts=True,
        num_devices=N_CORES,
    )

    xt_d = nc.dram_tensor("xt", [D_MODEL, S], bf16, kind="ExternalInput")
    w_d = {
        n: nc.dram_tensor(n, [D_MODEL, 128], bf16, kind="ExternalInput")
        for n in ("wq2", "wk2", "wv2")
    }
    wo_d = nc.dram_tensor("wo2", [64, 3 * D_MODEL], bf16, kind="ExternalInput")
    xqb_d = nc.dram_tensor("xqb", [D_MODEL, 2048], bf16, kind="ExternalInput")
    wqbs_d = nc.dram_tensor("wqbs", [D_MODEL, 64], bf16, kind="ExternalInput")
    cosqb_d = nc.dram_tensor("cosqb", [64, 2048], bf16, kind="ExternalInput")
    sinqb_d = nc.dram_tensor("sinqb", [64, 2048], bf16, kind="ExternalInput")
    maskh_d = nc.dram_tensor("maskh", [128, 1024], bf16, kind="ExternalInput")
    cosk_d = nc.dram_tensor("cosk", [128, S], bf16, kind="ExternalInput")
    sink_d = nc.dram_tensor("sink", [128, S], bf16, kind="ExternalInput")
    mask_d = nc.dram_tensor("maskab", [128, 512], bf16, kind="ExternalInput")
    out_d = nc.dram_tensor("out", [512, D_MODEL], f32, kind="ExternalOutput")
    o_part = nc.dram_tensor("o_part", [S, D_MODEL], bf16, kind="Internal")
    ors = [
        nc.dram_tensor(f"ors{j}", [(hi - lo) // 8, D_MODEL], bf16, kind="Internal")
        for j, (_, lo, hi) in enumerate(RS_CHUNKS)
    ]


    PAIRSWAP = [i ^ 1 for i in range(32)]

    with tile.TileContext(nc) as tc:
        with (
            tc.tile_pool(name="const", bufs=1) as cpool,
            tc.tile_pool(name="rope", bufs=4) as rpool,
            tc.tile_pool(name="expp", bufs=6) as epool,
            tc.tile_pool(name="norm", bufs=6) as npool,
            tc.tile_pool(name="rsrb", bufs=2) as rbpool,
            tc.tile_pool(name="ps", bufs=3, space="PSUM") as pspool,
            tc.tile_pool(name="ps_pv", bufs=2, space="PSUM") as psv,
        ):
            nc.gpsimd.load_library(library_config.attn)

            # ---- persistent tensors; first chunk's inputs loaded first ----
            def load_w(n):
                t = cpool.tile([128, EB * 128], bf16, tag=f"w_{n}", name=n)
                nc.sync.dma_start(
                    out=t[:].rearrange("p (e m) -> p e m", m=128),
                    in_=w_d[n][:].rearrange("(e p) m -> p e m", p=128),
                )
                return t

            def xt_load(lo, hi):
                nc.sync.dma_start(
                    out=xt_sb[:].rearrange("p (e s) -> p e s", s=S)[:, :, lo:hi],
                    in_=xt_d[:].rearrange("(e p) s -> p e s", p=128)[:, :, lo:hi],
                )

            wk_sb = load_w("wk2")
            xt_sb = cpool.tile([128, EB * S], bf16, tag="xt_sb")
            xt_load(0, 512)
            cosk_sb = cpool.tile([128, S], bf16, tag="cosk_sb")
            nc.sync.dma_start(out=cosk_sb[:, 0:512], in_=cosk_d[:, 0:512])
            sink_sb = cpool.tile([128, S], bf16, tag="sink_sb")
            nc.sync.dma_start(out=sink_sb[:, 0:512], in_=sink_d[:, 0:512])
            wq_sb = load_w("wq2")
            maskab = cpool.tile([128, 512], bf16, tag="maskab")
            nc.sync.dma_start(out=maskab[:], in_=mask_d[:])
            xqb_sb = cpool.tile([128, EB * 2048], bf16, tag="xqb_sb")
            nc.sync.dma_start(
                out=xqb_sb[:].rearrange("p (e s) -> p e s", s=2048)[:, :, 0:1024],
                in_=xqb_d[:].rearrange("(e p) s -> p e s", p=128)[:, :, 0:1024],
            )
            cosqb_sb = cpool.tile([128, 2048], bf16, tag="cosqb_sb")
            nc.sync.dma_start(out=cosqb_sb[64:128, :], in_=cosqb_d[:])
            sinqb_sb = cpool.tile([128, 2048], bf16, tag="sinqb_sb")
            nc.sync.dma_start(out=sinqb_sb[64:128, :], in_=sinqb_d[:])
            wqbs_sb = cpool.tile([128, EB * 64], bf16, tag="wqbs")
            nc.sync.dma_start(
                out=wqbs_sb[:].rearrange("p (e m) -> p e m", m=64),
                in_=wqbs_d[:].rearrange("(e p) m -> p e m", p=128),
            )
            maskh = cpool.tile([128, 1024], bf16, tag="maskh")
            nc.sync.dma_start(out=maskh[:], in_=maskh_d[:])
            wv_sb = load_w("wv2")
            wo_sb = cpool.tile([64, 3 * D_MODEL], bf16, tag="wo2")
            nc.sync.dma_start(out=wo_sb[:], in_=wo_d[:])
            xt_load(512, 1024)
            nc.sync.dma_start(out=cosk_sb[:, 512:S], in_=cosk_d[:, 512:S])
            nc.sync.dma_start(out=sink_sb[:, 512:S], in_=sink_d[:, 512:S])
            xt_load(1024, S)
            nc.sync.dma_start(
                out=xqb_sb[:].rearrange("p (e s) -> p e s", s=2048)[:, :, 1024:2048],
                in_=xqb_d[:].rearrange("(e p) s -> p e s", p=128)[:, :, 1024:2048],
            )

            qb = cpool.tile([128, 2048], bf16, tag="qb")
            attnb = cpool.tile([64, 2048], bf16, tag="attnb")
            ones64 = cpool.tile([65, 64], bf16, tag="ones64")
            nc.vector.memset(ones64[64:65, :], 1.0)
            qt = cpool.tile([128, S], bf16, tag="qt")
            ktc = [
                cpool.tile([128, 512], bf16, tag=f"kt{ch}", name=f"kt{ch}")
                for ch in range(N_CH)
            ]
            vc = [
                cpool.tile([128, 4 * VW], bf16, tag=f"va{ch}", name=f"va{ch}")
                for ch in range(N_CH)
            ]
            attn_sb = [cpool.tile([64, S], bf16, tag="attn0", name="attn0")]

            def rope(dst, src_ps, cos_ap, sin_ap):
                xb = rpool.tile([128, 512], bf16, tag="rope_x")
                nc.vector.tensor_copy(xb[:], src_ps[:])
                sh = rpool.tile([128, 512], bf16, tag="rope_sh")
                nc.vector.stream_shuffle(sh[:], xb[:], PAIRSWAP)
                nc.vector.tensor_mul(xb[:], xb[:], cos_ap)
                nc.vector.tensor_mul(sh[:], sh[:], sin_ap)
                nc.vector.tensor_add(dst, xb[:], sh[:])

            def attention_tile(slot, T):
                """Score/exp/PV matmuls for one (head-slot, 256-row q-tile).

                Software-pipelined: group g+1's score matmuls are issued
                before group g's PV matmuls so the PE never sits behind the
                exp. Normalization is issued later (see finalize_tile)."""
                ro = 64 * slot
                qslice = qt[ro:ro + 64, T * 256:T * 256 + 256]
                pv = psv.tile([65, 256], f32, tag="ps_pv")
                groups = [(pg, 2 if pg + 1 <= T else 1) for pg in range(0, T + 1, 2)]

                def issue_pv(pg, w, et):
                    for pi in range(w):
                        p = pg + pi
                        for j in range(2):
                            t = 2 * p + j
                            nc.tensor.matmul(
                                pv[:],
                                vc[t // 4][:, (t % 4) * VW + slot * 65:(t % 4) * VW + slot * 65 + 65],
                                et[:, (2 * pi + j) * 256:(2 * pi + j + 1) * 256],
                                start=(p == 0 and j == 0),
                                stop=(p == T and j == 1),
                            )

                prev = None
                for pg, w in groups:
                    sc = pspool.tile([128, 1024], f32, tag="ps")
                    for pi in range(w):
                        for j in range(2):
                            t = 2 * (pg + pi) + j
                            nc.tensor.matmul(
                                sc[:, (2 * pi + j) * 256:(2 * pi + j + 1) * 256],
                                ktc[t // 4][ro:ro + 64, (t % 4) * 128:(t % 4) * 128 + 128],
                                qslice,
                                start=True,
                                stop=True,
                            )
                    et = epool.tile([128, 1024], bf16, tag="et")
                    nc.scalar.activation(
                        et[:, 0:512 * w], sc[:, 0:512 * w], Exp, bias=0.0, scale=0.125
                    )
                    if pg + w - 1 == T:  # group holds the diagonal pair
                        off = 512 * (w - 1)
                        nc.vector.tensor_mul(
                            et[:, off:off + 512], et[:, off:off + 512], maskab[:]
                        )
                    if prev is not None:
                        issue_pv(*prev)
                    prev = (pg, w, et)
                issue_pv(*prev)
                # reduce + reciprocal now; broadcast and normalize deferred
                pvs = npool.tile([65, 256], f32, tag="pvs")
                nc.vector.tensor_copy(pvs[:], pv[:])
                rrow = npool.tile([65, 256], bf16, tag="rrow")
                with nc.allow_low_precision(reason="bf16 softmax denominators"):
                    nc.vector.reciprocal(rrow[64:65, :], pvs[64:65, :])
                return pvs, rrow

            def b_tile(m):
                """Head-B attention for the core's m-th q-tile (physical
                q-tile 2m+parity, supplied pre-permuted in qb). Runs 2m+2
                pairs; the parity-dependent [128,1024] maskh input handles
                the diagonal and the padded pair in the last group."""
                qsl = qb[64:128, m * 256:m * 256 + 256]
                pv = psv.tile([65, 256], f32, tag="ps_pv")
                npairs = 2 * m + 2

                def issue_pvb(pg, et):
                    for pi in range(2):
                        p = pg + pi
                        for j in range(2):
                            t = 2 * p + j
                            nc.tensor.matmul(
                                pv[:],
                                vc[t // 4][:, (t % 4) * VW + 65:(t % 4) * VW + 130],
                                et[:, (2 * pi + j) * 256:(2 * pi + j + 1) * 256],
                                start=(p == 0 and j == 0),
                                stop=(p == npairs - 1 and j == 1),
                            )

                prev = None
                for pg in range(0, npairs, 2):
                    sc = pspool.tile([128, 1024], f32, tag="ps")
                    for pi in range(2):
                        for j in range(2):
                            t = 2 * (pg + pi) + j
                            nc.tensor.matmul(
                                sc[:, (2 * pi + j) * 256:(2 * pi + j + 1) * 256],
                                ktc[t // 4][64:128, (t % 4) * 128:(t % 4) * 128 + 128],
                                qsl,
                                start=True,
                                stop=True,
                            )
                    et = epool.tile([128, 1024], bf16, tag="et")
                    nc.scalar.activation(
                        et[:], sc[:], Exp, bias=0.0, scale=0.125
                    )
                    if pg + 2 >= npairs:  # last group: diagonal + padding
                        nc.vector.tensor_mul(et[:], et[:], maskh[:])
                    if prev is not None:
                        issue_pvb(*prev)
                    prev = (pg, et)
                issue_pvb(*prev)
                pvs = npool.tile([65, 256], f32, tag="pvs")
                nc.vector.tensor_copy(pvs[:], pv[:])
                rrow = npool.tile([65, 256], bf16, tag="rrow")
                with nc.allow_low_precision(reason="bf16 softmax denominators"):
                    nc.vector.reciprocal(rrow[64:65, :], pvs[64:65, :])
                return pvs, rrow

            def normalize(pvs, rrow, dst):
                rb = psv.tile([65, 256], f32, tag="ps_pv")
                nc.tensor.matmul(
                    rb[0:64, :], ones64[64:65, :], rrow[64:65, :],
                    start=True, stop=True,
                )
                nc.vector.tensor_mul(dst, pvs[0:64, :], rb[0:64, :])

            def finalize_tile(T, handles):
                """Deferred normalize (broadcast via K=1 matmul) + output
                projection for q-tile T; issued one tile later so the
                reciprocal is ready and the PE never waits."""
                pvs, rrow = handles
                normalize(pvs, rrow, attn_sb[0][0:64, T * 256:T * 256 + 256])
                o_proj_pair(T)

            def finalize_btile(m, handles):
                pvs, rrow = handles
                normalize(pvs, rrow, attnb[0:64, m * 256:m * 256 + 256])

            def o_proj_pair(T):
                osb = rbpool.tile([128, 2 * D_MODEL], bf16, tag="osb")
                for half in range(2):
                    qtl = 2 * T + half
                    pot = pspool.tile([128, 1024], f32, tag="ps")
                    po = pot[:, 0:512]
                    po2 = pot[:, 512:768]
                    lhsT = attn_sb[0][0:64, qtl * 128:qtl * 128 + 128]
                    # head-B contribution rides the same accumulation: the
                    # weight slice is real or zero depending on whether this
                    # core's parity owns q-tile T (selected by host data)
                    wb = D_MODEL + (T % 2) * D_MODEL
                    lhsB = attnb[0:64, (T // 2) * 256 + (qtl % 2) * 128:(T // 2) * 256 + (qtl % 2) * 128 + 128]
                    nc.tensor.matmul(po, lhsT, wo_sb[:, 0:512], start=True, stop=False)
                    nc.tensor.matmul(
                        po, lhsB, wo_sb[:, wb:wb + 512], start=False, stop=True
                    )
                    nc.tensor.matmul(
                        po2, lhsT, wo_sb[:, 512:768], start=True, stop=False
                    )
                    nc.tensor.matmul(
                        po2, lhsB, wo_sb[:, wb + 512:wb + 768], start=False, stop=True
                    )
                    nc.vector.tensor_copy(
                        osb[:, half * D_MODEL:(half + 1) * D_MODEL],
                        pot[:, 0:768],
                    )
                last_opart[0] = nc.sync.dma_start(
                    out=o_part[T * 256:(T + 1) * 256, :].rearrange(
                        "(h p) d -> p h d", p=128
                    ),
                    in_=osb[:].rearrange("p (h d) -> p h d", d=D_MODEL),
                )

            def fire_rs(j):
                _, lo, hi = RS_CHUNKS[j]
                nc.gpsimd.collective_compute(
                    "ReduceScatter",
                    mybir.AluOpType.add,
                    replica_groups=[list(range(N_CORES))],
                    ins=[o_part[lo:hi, :]],
                    outs=[ors[j][:]],
                )

            def readback(j):
                _, lo, hi = RS_CHUNKS[j]
                shard = (hi - lo) // 8
                for b in range(0, shard, 128):
                    bb = min(128, shard - b)
                    rt = rbpool.tile([128, D_MODEL], bf16, tag="rt")
                    rd = nc.sync.dma_start(out=rt[0:bb, :], in_=ors[j][b:b + bb, :])
                    # keep readback DMAs after all o_part writes so the
                    # round-robin DMA-queue counts of collective waits never
                    # include collective-dependent transfers
                    if last_opart[0] is not None:
                        add_dep_helper(
                            rd.ins, last_opart[0].ins, sync=True,
                            reason="readback after o_part stream",
                        )
                    rtf = rbpool.tile([128, D_MODEL], f32, tag="rtf")
                    nc.vector.tensor_copy(rtf[0:bb, :], rt[0:bb, :])
                    nc.sync.dma_start(
                        out=out_d[RS_OUT_OFF[j] + b:RS_OUT_OFF[j] + b + bb, :],
                        in_=rtf[0:bb, :],
                    )

            # ---- main loop: projection chunk ch, then attention q-tiles;
            # tile finalization (normalize + o_proj + RS) runs one tile behind
            rs_next = 0
            last_opart = [None]
            b_done = False
            pending = None  # (kind, idx, handles)

            def do_proj_kq(ch):
                def xt_t_slice(eb, lo, hi):
                    return xt_sb[:, eb * S + ch * 512 + lo:eb * S + ch * 512 + hi]
                ck = cosk_sb[:, ch * 512:(ch + 1) * 512]
                sk = sink_sb[:, ch * 512:(ch + 1) * 512]

                # K^T and Q^T share one PSUM tile; RoPE applied to both
                psKQ = pspool.tile([128, 1024], f32, tag="ps")
                for eb in range(EB):
                    nc.tensor.matmul(
                        psKQ[:, 0:512],
                        wk_sb[:, eb * 128:(eb + 1) * 128],
                        xt_t_slice(eb, 0, 512),
                        start=(eb == 0),
                        stop=(eb == EB - 1),
                    )
                for eb in range(EB):
                    nc.tensor.matmul(
                        psKQ[:, 512:1024],
                        wq_sb[:, eb * 128:(eb + 1) * 128],
                        xt_t_slice(eb, 0, 512),
                        start=(eb == 0),
                        stop=(eb == EB - 1),
                    )
                rope(ktc[ch][:], psKQ[:, 0:512], ck, sk)
                rope(qt[:, ch * 512:(ch + 1) * 512], psKQ[:, 512:1024], ck, sk)

            def do_proj_v(ch):
                def xt_t_slice(eb, lo, hi):
                    return xt_sb[:, eb * S + ch * 512 + lo:eb * S + ch * 512 + hi]
                # V (natural layout, interleaved ones column per head):
                # 4 s-tiles accumulate into one PSUM tile
                psV4 = pspool.tile([128, 1024], f32, tag="ps")
                for stl in range(4):
                    for eb in range(EB):
                        nc.tensor.matmul(
                            psV4[:, stl * 256:stl * 256 + 128],
                            xt_t_slice(eb, stl * 128, stl * 128 + 128),
                            wv_sb[:, eb * 128:(eb + 1) * 128],
                            start=(eb == 0),
                            stop=(eb == EB - 1),
                        )
                for stl in range(4):
                    vtile = vc[ch][:, stl * VW:(stl + 1) * VW].rearrange(
                        "p (h d) -> p h d", d=65
                    )
                    nc.vector.memset(vtile[:, :, 64:65], 1.0)
                    nc.vector.tensor_copy(
                        vtile[:, :, 0:64],
                        psV4[:, stl * 256:stl * 256 + 128].rearrange(
                            "p (h d) -> p h d", d=64
                        ),
                    )

            def do_proj_qb(q2):
                """Project + RoPE a 512-col quarter of the permuted head-B
                q-columns into qb rows 64:128. The RoPE pair-swap comes from
                a second projection against host-swapped weights, so every
                vector op runs full-width and no partition shuffle is
                needed (rows 0:64 carry unused garbage)."""
                qs = slice(q2 * 512, q2 * 512 + 512)
                psB = pspool.tile([128, 1024], f32, tag="ps")
                for eb in range(EB):
                    nc.tensor.matmul(
                        psB[64:128, 0:512],
                        wq_sb[:, eb * 128 + 64:(eb + 1) * 128],
                        xqb_sb[:, eb * 2048 + q2 * 512:eb * 2048 + q2 * 512 + 512],
                        start=(eb == 0),
                        stop=(eb == EB - 1),
                    )
                for eb in range(EB):
                    nc.tensor.matmul(
                        psB[64:128, 512:1024],
                        wqbs_sb[:, eb * 64:(eb + 1) * 64],
                        xqb_sb[:, eb * 2048 + q2 * 512:eb * 2048 + q2 * 512 + 512],
                        start=(eb == 0),
                        stop=(eb == EB - 1),
                    )
                xb = rpool.tile([128, 512], bf16, tag="rope_x")
                nc.vector.tensor_copy(xb[64:128, :], psB[64:128, 0:512])
                sh = rpool.tile([128, 512], bf16, tag="rope_sh")
                nc.vector.tensor_copy(sh[64:128, :], psB[64:128, 512:1024])
                nc.vector.tensor_mul(xb[64:128, :], xb[64:128, :], cosqb_sb[64:128, qs])
                nc.vector.tensor_mul(sh[64:128, :], sh[64:128, :], sinqb_sb[64:128, qs])
                nc.vector.tensor_add(qb[64:128, qs], xb[64:128, :], sh[64:128, :])

            # projections run one chunk ahead, split and issued mid-chunk so
            # the attention tiles keep the Act engine fed at boundaries
            do_proj_kq(0)
            do_proj_v(0)
            do_proj_qb(0)
            do_proj_kq(1)
            do_proj_v(1)

            def pop_pending():
                nonlocal pending, rs_next, b_done
                if pending is None:
                    return
                kind, idx, ph = pending
                if kind == "A":
                    finalize_tile(idx, ph)
                    if (
                        with_rs
                        and rs_next < len(RS_CHUNKS)
                        and RS_CHUNKS[rs_next][0] == idx
                    ):
                        fire_rs(rs_next)
                        rs_next += 1
                else:
                    finalize_btile(idx, ph)
                pending = None

            for ch in range(N_CH):
                bh = b_tile(ch)
                pop_pending()
                pending = ("B", ch, bh)
                for T in (2 * ch, 2 * ch + 1):
                    handles = attention_tile(0, T)
                    pop_pending()
                    pending = ("A", T, handles)
                    if ch + 2 < N_CH:
                        if T == 2 * ch:
                            do_proj_kq(ch + 2)
                        else:
                            do_proj_v(ch + 2)
                    if ch in (1, 3, 5) and T == 2 * ch:
                        do_proj_qb((ch + 1) // 2)
            pop_pending()
            while with_rs and rs_next < len(RS_CHUNKS):
                fire_rs(rs_next)
                rs_next += 1
            if with_rs:
                for j in range(len(RS_CHUNKS)):
                    readback(j)


    nc.compile()
    return nc


_PROGRAM = None


def _get_program():
    global _PROGRAM
    if _PROGRAM is None:
        _PROGRAM = build_program()
    return _PROGRAM


def host_prep(in_features, token_positions, q_proj, k_proj, v_proj, o_proj):
    """Build the 8 per-core input maps."""
    x = np.asarray(in_features, np.float32).reshape(S, D_MODEL)
    tp = np.asarray(token_positions)
    qp = np.asarray(q_proj, np.float32)
    kp = np.asarray(k_proj, np.float32)
    vp = np.asarray(v_proj, np.float32)
    op = np.asarray(o_proj, np.float32)

    xt_bf = np.ascontiguousarray(x.T).astype(BF16)      # [768, 4096]
    wqT = np.ascontiguousarray(qp.T)                    # [in 768, out 768]
    wkT = np.ascontiguousarray(kp.T)
    wvT = np.ascontiguousarray(vp.T)
    opT = np.ascontiguousarray(op.T)                    # [in-dk 768, out 768]

    inv_freq = 1.0 / THETA ** (np.arange(0, DK, 2, dtype=np.float32) / DK)
    pos = np.clip(tp.astype(np.float32), 0, MAX_SEQ_LEN - 1)
    freq = pos[:, None] * inv_freq[None, :]             # [S, 32]
    cos_t, sin_t = np.cos(freq), np.sin(freq)

    r = np.arange(128)
    fidx = (r % 64) // 2
    sign = np.where(r % 2 == 0, -1.0, 1.0).astype(np.float32)
    cos128 = cos_t[:, fidx].T.astype(BF16)              # [128, S]
    sin128 = (sin_t[:, fidx].T * sign[:, None]).astype(BF16)

    ki = np.arange(128)[:, None]
    qi = np.arange(256)[None, :]
    mask_a = (ki <= qi).astype(np.float32)
    mask_b = (ki + 128 <= qi).astype(np.float32)
    maskab = np.concatenate([mask_a, mask_b], axis=1).astype(BF16)

    ones512 = np.ones((128, 512), np.float32)
    zeros512 = np.zeros((128, 512), np.float32)

    in_maps = []
    for c in range(N_CORES):
        hA = c
        hB = 8 + c // 2
        p = c % 2

        def wslice(wT):
            out = np.empty((D_MODEL, 128), np.float32)
            out[:, 0:64] = wT[:, hA * 64:(hA + 1) * 64]
            out[:, 64:128] = wT[:, hB * 64:(hB + 1) * 64]
            return out.astype(BF16)

        wo2 = np.zeros((64, 3 * D_MODEL), np.float32)
        wo2[:, 0:D_MODEL] = opT[hA * 64:(hA + 1) * 64, :]
        # slot 1 = used for even q-tiles, slot 2 = odd q-tiles; only the
        # parity this core owns carries real head-B weights
        wo2[:, (1 + p) * D_MODEL:(2 + p) * D_MODEL] = opT[hB * 64:(hB + 1) * 64, :]

        wqB = wqT[:, hB * 64:(hB + 1) * 64]
        swap = np.arange(64) ^ 1
        wqbs = np.ascontiguousarray(wqB[:, swap]).astype(BF16)

        qcolsB = np.concatenate(
            [np.arange(256 * (2 * m + p), 256 * (2 * m + p) + 256) for m in range(8)]
        )
        maskab_f = maskab.astype(np.float32)
        if p == 0:
            maskh = np.concatenate([maskab_f, zeros512], axis=1)
        else:
            maskh = np.concatenate([ones512, maskab_f], axis=1)

        in_maps.append(
            {
                "xt": xt_bf,
                "xqb": np.ascontiguousarray(xt_bf[:, qcolsB]),
                "wqbs": wqbs,
                "wq2": wslice(wqT),
                "wk2": wslice(wkT),
                "wv2": wslice(wvT),
                "wo2": wo2.astype(BF16),
                "cosk": cos128,
                "sink": sin128,
                "cosqb": np.ascontiguousarray(cos128[0:64, qcolsB]),
                "sinqb": np.ascontiguousarray(sin128[0:64, qcolsB]),
                "maskab": maskab,
                "maskh": maskh.astype(BF16),
            }
        )
    return in_maps


def assemble_output(results):
    out = np.empty((1, S, D_MODEL), np.float32)
    for c in range(N_CORES):
        r = np.asarray(results[c]["out"], np.float32)
        for j, (_, lo, hi) in enumerate(RS_CHUNKS):
            shard = (hi - lo) // 8
            oo = RS_OUT_OFF[j]
            out[0, lo + shard * c:lo + shard * (c + 1)] = r[oo:oo + shard]
    return out


def kernel(**inputs):
    from concourse.bass_utils import run_bass_kernel_spmd

    nc = _get_program()
    in_maps = host_prep(**inputs)
    res = run_bass_kernel_spmd(nc, in_maps, list(range(N_CORES)))
    return assemble_output(res.results)


if __name__ == "__main__":
    nc = build_program()
    print("program built and compiled")

